# revision 19
# baseline (speedup 1.0000x reference)
"""ConvR (dense_cnn) Trainium2 kernel — 8-core vocab/tensor-parallel, f16.

Strategy (per sharding hint): entity-embedding table and output scores are
column-sharded across the 8 cores; the small conv/fc path is replicated on
every core (each core computes the full 256-sample hidden, then scores its
12500-entity shard).

Key design points vs the old f32 version:
  - every matmul operand is f16 (same PE rate as bf16, 8x finer mantissa) (fp32 matmuls are decomposed 2x by the
    compiler and stream at 1/4 rate; bf16 also halves HBM traffic)
  - conv: 4 samples per matmul via a block-diagonal rhs (one LDWEIGHTS per
    4 samples instead of per sample -- the old kernel was LDW-bound)
  - embT is loaded in chunks on 16-engine queues (the old kernel put one
    5 MB DMA on the scalar HWDGE queue, which serialized on a single SDMA
    engine at ~26 GB/s for ~195 us)
  - scores go out as uint8 (sigmoid in [0,1] scaled by 255+0.499 on DVE in
    4x mode, cast to u8 during the SWDGE out-DMA); host divides by 255
  - m0/m1 pipelining: scoring of samples 0-127 (PE -> ACT sigmoid -> DVE
    x255 -> gpsimd cast-DMA) overlaps conv+fc of samples 128-255
  - ACT runs ONLY Sigmoid (conv/fc relu-evacs on DVE) so there is a single
    activation-table load, warmed up during the input-DMA phase
"""
import os
import sys

sys.path.insert(0, "/opt/trn_rl_repo")

import numpy as np
from contextlib import ExitStack

B = 256          # batch
E = 100          # embedding dim
NE = 100000      # entities
NCORES = 8
SH = NE // NCORES   # 12500 entities per core
EPS = 1e-5
G = 64           # conv groups of 4 samples
NCH = 512        # scoring N-chunk (one PSUM bank of f32)
# per-m scoring sigmoid groups: alternating 4-bank / 2-bank + 212 tail
SGROUPS = [4 * NCH, 2 * NCH] * 4 + [SH - 24 * NCH]   # sums to 12500

OUT_U8 = True    # False -> ship f16 sigmoid scores (fallback)

_CACHE = {}


def _build():
    import concourse.bass as bass  # noqa: F401
    import concourse.tile as tile
    from concourse import bacc, mybir

    f32 = mybir.dt.float32
    f16 = mybir.dt.float16
    u8 = mybir.dt.uint8
    AF = mybir.ActivationFunctionType
    OP = mybir.AluOpType

    nc = bacc.Bacc("TRN2", target_bir_lowering=False, debug=False,
                   num_devices=NCORES)

    # r3 (filters) and p3 (block-diag patches) interleaved per group so one
    # chunk DMA has large per-partition-row descriptors: cv[:, g*244+:100] is
    # the group-g lhsT, cv[:, g*244+100:+144] the group-g rhs
    cv_d = nc.dram_tensor("cv", [128, G * 244], f16, kind="ExternalInput").ap()
    w3_d = nc.dram_tensor("w3", [100, 3600], f16, kind="ExternalInput").ap()
    bc_d = nc.dram_tensor("bc", [100, 2], f32, kind="ExternalInput").ap()
    ones_d = nc.dram_tensor("ones", [1, B], f16, kind="ExternalInput").ap()
    embT_d = nc.dram_tensor("embT", [101, SH], f16, kind="ExternalInput").ap()
    out_dt = u8 if OUT_U8 else f16
    scores_d = nc.dram_tensor("scores", [B, SH], out_dt,
                              kind="ExternalOutput").ap()

    with tile.TileContext(nc) as tc, ExitStack() as ctx:
        cpool = ctx.enter_context(tc.tile_pool(name="const", bufs=1))

        bc_t = cpool.tile([100, 2], f32, tag="bc")
        b1_t = bc_t[:, 0:1]
        b2_t = bc_t[:, 1:2]
        cv_t = cpool.tile([128, G * 244], f16, tag="cv")
        w3_t = cpool.tile([100, 3600], f16, tag="w3")
        X_t = cpool.tile([100, 36 * B], f16, tag="X")
        hT_t = cpool.tile([101, B], f16, tag="hT")
        embT_t = cpool.tile([101, SH], f16, tag="embT")
        sig_warm = cpool.tile([1, 1], f16, tag="sigw")

        # ---- input DMAs: ALL bulk on the sync/HWDGE queue (measured ~290
        # GB/s with ~1MB chunks; the gpsimd/SWDGE queue only sustains ~100
        # GB/s -- putting embT there starved the scoring chain until 50us).
        # FIFO order = priority order, scheduled to each consumer's deadline.
        nc.sync.dma_start(bc_t[:], bc_d[:])

        def cv_chunk(ci):
            a, b = ci * 16 * 244, (ci + 1) * 16 * 244
            nc.sync.dma_start(cv_t[:, a:b], cv_d[:, a:b])

        EC = [0, 3072, 6144, 9216, 12288, SH]   # embT chunk cols

        def emb_chunk(c):
            nc.sync.dma_start(embT_t[:, EC[c]:EC[c + 1]],
                              embT_d[:, EC[c]:EC[c + 1]])

        cv_chunk(0)
        cv_chunk(1)                       # conv m0 complete
        nc.sync.dma_start(w3_t[:], w3_d[:])
        emb_chunk(0)
        emb_chunk(1)
        cv_chunk(2)
        emb_chunk(2)
        emb_chunk(3)
        cv_chunk(3)
        emb_chunk(4)
        nc.gpsimd.dma_start(hT_t[100:101, :], ones_d[:])

        # preload the Sigmoid activation-table set while DMAs run, so the
        # ~2.7us table load is off the critical path
        nc.scalar.activation(sig_warm[:], bc_t[0:1, 0:1], AF.Sigmoid)

        # PSUM pools: conv/fc share one 2-bank tag; scoring uses 4+2 banks.
        pconv = ctx.enter_context(
            tc.tile_pool(name="pconv", bufs=2, space="PSUM"))
        psc = ctx.enter_context(
            tc.tile_pool(name="psc", bufs=1, space="PSUM"))

        sigp = ctx.enter_context(tc.tile_pool(name="sigp", bufs=3))

        # ---- PE warm-up spin: ~20 back-to-back dummy matmuls on a zeroed
        # tile trip the HAM clock gate from 4/8 (1.2 GHz) to 8/8 (2.4 GHz)
        # ~3.4us in; the spin runs during the input-DMA phase.  Without it
        # the whole kernel's matmuls run at half clock (measured).  The
        # memset runs on DVE, which is idle until the first conv evac.
        wz = cpool.tile([128, 512], f16, tag="wz")
        nc.vector.memset(wz[:], 0.0)
        pw = psc.tile([128, 4 * NCH], f32, tag="psc_a", name="pw")
        for i in range(20):
            nc.tensor.matmul(pw[:, (i % 4) * NCH:(i % 4 + 1) * NCH],
                             wz[:, 0:128], wz[:],
                             start=True, stop=True)

        def conv_tiles(m):
            """per-PSUM-tile closures for conv of samples 128m..128m+127
            (groups 32m..+32, 3 groups of 4 samples per tile).  Evac on DVE
            (relu+bias via tensor_scalar); ACT is reserved for Sigmoid."""
            g0 = 32 * m
            tiles = [(g0 + t * 3, min(3, 32 - t * 3)) for t in range(11)]

            # X layout is hw-major: X[c, hw*256 + s] so the fc rhs slices are
            # contiguous; the (s,hw)->(hw,s) transpose happens in the evac AP
            Xh = X_t[:].rearrange("p (hw s) -> p hw s", s=B)

            def make(gs, ng):
                def emit():
                    pt = pconv.tile([100, 432], f32, tag="pconv", name="pt")
                    for j in range(ng):
                        g = gs + j
                        nc.tensor.matmul(
                            pt[:, j * 144:(j + 1) * 144],
                            cv_t[:, g * 244:g * 244 + 100],
                            cv_t[:, g * 244 + 100:(g + 1) * 244],
                            start=True, stop=True)
                    src = pt[:, 0:ng * 144].rearrange("p (s hw) -> p hw s",
                                                      hw=36)
                    nc.vector.tensor_scalar(
                        Xh[:, :, gs * 4:(gs + ng) * 4], src,
                        b1_t, 0.0, OP.add, OP.max)
                return emit
            return [make(gs, ng) for gs, ng in tiles]

        def fc_half(m):
            pf = pconv.tile([100, 128], f32, tag="pconv", name="pf")
            for hw in range(36):
                nc.tensor.matmul(
                    pf[:],
                    w3_t[:, hw * 100:(hw + 1) * 100],
                    X_t[:, hw * B + m * 128:hw * B + (m + 1) * 128],
                    start=(hw == 0), stop=(hw == 35))
            nc.vector.tensor_scalar(hT_t[0:100, m * 128:(m + 1) * 128],
                                    pf[:], b2_t, 0.0, OP.add, OP.max)

        def score_group(m, col0, width, interleave=()):
            """one sigmoid group: ceil(width/NCH) matmuls -> ACT sigmoid ->
            DVE x255 (+0.499 so the trunc cast rounds) -> gpsimd cast-DMA."""
            banks = (width + NCH - 1) // NCH
            tag = "psc_a" if banks > 2 else "psc_b"
            ps = psc.tile([128, banks * NCH], f32, tag=tag, name="ps")
            for c in range(banks):
                n = min(NCH, width - c * NCH)
                nc.tensor.matmul(
                    ps[:, c * NCH:c * NCH + n],
                    hT_t[:, m * 128:(m + 1) * 128],
                    embT_t[:, col0 + c * NCH:col0 + c * NCH + n],
                    start=True, stop=True)
            for work in interleave:
                work()
            sg = sigp.tile([128, 2048], f16, tag="sig", name="sg")
            sview = sg[:, 0:width]
            nc.scalar.activation(sview, ps[:, 0:width], AF.Sigmoid)
            if OUT_U8:
                nc.vector.tensor_scalar(sview, sview, 255.0, 0.499,
                                        OP.mult, OP.add)
                nc.gpsimd.dma_start(
                    scores_d[m * 128:(m + 1) * 128, col0:col0 + width], sview)
            else:
                nc.sync.dma_start(
                    scores_d[m * 128:(m + 1) * 128, col0:col0 + width], sview)

        # ---- m0: conv + fc
        for emit in conv_tiles(0):
            emit()
        fc_half(0)

        # ---- scoring m0, with conv/fc m1 interleaved between groups.
        # 4-bank groups get 2 conv tiles, 2-bank groups 1; fc after all conv.
        m1work = conv_tiles(1) + [lambda: fc_half(1)]
        # conv-m1 sits in the PE FIFO ahead of later scoring matmuls, so it
        # must not be queued before its input chunks have landed: spread it
        # over the late m0 score-groups to match the DMA supply schedule
        share = [0, 0, 0, 0, 0, 3, 3, 3, 2]  # per score-group m1 tiles
        col = 0
        wi = 0
        for gi, width in enumerate(SGROUPS):
            take = m1work[wi:wi + share[gi]]
            wi += share[gi]
            score_group(0, col, width, interleave=take)
            col += width
        for work in m1work[wi:]:
            work()

        # ---- scoring m1
        col = 0
        for width in SGROUPS:
            score_group(1, col, width)
            col += width

    nc.compile()
    return nc


def host_prep(inputs):
    f = {k: np.asarray(v) for k, v in inputs.items()}
    e1 = f['e1'].astype(np.int64)
    rel = f['rel'].astype(np.int64)
    e1e = np.ascontiguousarray(f['emb_e'][e1]).astype(np.float32)    # (B, 100)
    rg = np.ascontiguousarray(f['emb_rel'][rel]).astype(np.float32)  # (B, 2500)

    a0 = float(f['bn0_g'][0] / np.sqrt(f['bn0_v'][0] + EPS))
    b0 = float(f['bn0_b'][0] - f['bn0_m'][0] * a0)
    A1 = (f['bn1_g'] / np.sqrt(f['bn1_v'] + EPS)).astype(np.float32)
    B1 = (f['bn1_b'] - f['bn1_m'] * A1).astype(np.float32)
    s_rel = (f['bn_rel_g'] / np.sqrt(f['bn_rel_v'] + EPS)).astype(np.float32)
    t_rel = (f['bn_rel_b'] - f['bn_rel_m'] * s_rel).astype(np.float32)
    s_rel2 = s_rel * np.repeat(A1, 25)
    t_rel2 = t_rel * np.repeat(A1, 25)
    A2 = (f['bn2_g'] / np.sqrt(f['bn2_v'] + EPS)).astype(np.float32)
    B2p = ((f['fc_b'] - f['bn2_m']) * A2 + f['bn2_b']).astype(np.float32)

    bf = np.float16

    # filters, bn1-scale folded: rn[s, c*25+k].  conv lhsT layout:
    # r3[32*i + k, g*100 + c] = rn[4g+i, c, k]  (rows 25..31 of each 32-block
    # are zero; they pair with the zero rows of the block-diagonal rhs)
    rn = rg * s_rel2[None, :] + t_rel2[None, :]
    rn4 = rn.reshape(G, 4, 100, 25)                  # [g, i, c, k]
    r3 = np.zeros((4, 32, G, 100), np.float32)
    r3[:, :25] = rn4.transpose(1, 3, 0, 2)           # [i, k, g, c]
    r3 = r3.reshape(128, G, 100)

    # BN0-normalized patches, block-diagonal rhs:
    # p3[32*i + k, g*144 + 36*i + hw] = patch[4g+i, k, hw], zero elsewhere
    x0 = e1e * a0 + b0
    xg = x0.reshape(B, 10, 10)
    win = np.lib.stride_tricks.sliding_window_view(xg, (5, 5), axis=(1, 2))
    # win: [B, 6, 6, 5, 5] -> patch[s, k=ky*5+kx, hw=oy*6+ox]
    patch = win.transpose(0, 3, 4, 1, 2).reshape(B, 25, 36)
    p4 = patch.reshape(G, 4, 25, 36)                 # [g, i, k, hw]
    p3 = np.zeros((4, 32, G, 4, 36), np.float32)
    for i in range(4):
        p3[i, :25, :, i, :] = p4[:, i].transpose(1, 0, 2)
    p3 = p3.reshape(128, G, 144)
    cv = np.ascontiguousarray(
        np.concatenate([r3, p3], axis=2).reshape(128, G * 244))

    # fc weights, bn2-scale folded: w3[c, hw*100 + j] = (fc_w*A2)[c*36+hw, j]
    w3 = np.ascontiguousarray(
        (f['fc_w'].astype(np.float32) * A2[None, :]).reshape(100, 3600))

    embT = np.ascontiguousarray(np.concatenate(
        [f['emb_e'].T, f['bias'][None, :]], 0).astype(np.float32))  # (101, NE)

    common = dict(
        cv=cv.astype(bf), w3=w3.astype(bf),
        bc=np.ascontiguousarray(np.stack([B1, B2p], axis=1)).astype(np.float32),
        ones=np.ones((1, B), bf))
    in_maps = []
    for m in range(NCORES):
        d = dict(common)
        d['embT'] = np.ascontiguousarray(
            embT[:, m * SH:(m + 1) * SH]).astype(bf)
        in_maps.append(d)
    return in_maps


def _get_nc():
    if 'nc' not in _CACHE:
        _CACHE['nc'] = _build()
    return _CACHE['nc']


def kernel(**inputs):
    from concourse import bass_utils
    from concourse.bass_interp import get_hw_module

    nc = _get_nc()
    in_maps = host_prep(inputs)

    kwargs = {}
    trace_dir = os.environ.get("CONVR_TRACE_DIR")
    if trace_dir:
        kwargs.update(tmpdir=trace_dir, trace=True)

    old_m = nc.m
    nc.m = get_hw_module(nc.m)
    try:
        res = bass_utils.run_bass_kernel_spmd(
            nc, in_maps, core_ids=list(range(NCORES)), **kwargs)
    finally:
        nc.m = old_m
    _CACHE['last_result'] = res

    out = np.empty((B, NE), np.float32)
    for m in range(NCORES):
        s = res.results[m]['scores']
        if OUT_U8:
            out[:, m * SH:(m + 1) * SH] = s.astype(np.float32) * (1.0 / 255.0)
        else:
            out[:, m * SH:(m + 1) * SH] = s.astype(np.float32)
    return out
